# revision 25
# baseline (speedup 1.0000x reference)
"""Additive (Bahdanau) attention on 8 Trainium2 NeuronCores.

  q = queries @ W_q.T            [B,Q,H]
  k = keys    @ W_k.T            [B,K,H]
  scores[b,q,k] = sum_h w_v[h] * tanh(q[b,q,h] + k[b,k,h])
  out = softmax_k(scores) @ values

Sharding: data-parallel over batch, B=16 -> 2 batches per core.

Per-core dataflow (h on partitions, 2 h-halves of 128):
  - host pre-transposes queries/keys to [d, seq] so the projections are
    plain matmuls producing qT/kT[h, seq] tiles.
  - DVE tensor_scalar_add broadcasts one q-column over kT -> S[h, NQ*K]
    blocks (fp16 streamed operand, fp32 per-partition scalar).
  - one big ACT tanh per S block (the 268M-element tanh is the
    bottleneck engine: 1 elem/lane/cycle at 1.2 GHz regardless of
    dtype; large free dims amortize the ~224-cycle ACT overhead).
  - w_v reduction on TensorE: stationary [128,32] = w_v chunk in column
    (q%32), zeros elsewhere; each q's matmul writes one row of a
    32-aligned PSUM column-group, PSUM accumulation over h assembles
    scores[q, k] in natural layout.
  - loop is q-block-outer / h-half-inner so each 32-row column group
    finalizes early and softmax/AV/output DMA overlap with later tanh
    blocks.
  - exp (same ACT table set as tanh -> no table reload) with accum_out
    row-sums; softmax max-subtraction is skipped (|scores| <=
    sum|w_v| ~ 13, safe in fp32 exp); normalization is folded into the
    PSUM->SBUF copy after the attn @ values matmul (exp scores get
    transposed k-major via TensorE transpose).
"""

import sys

sys.path.insert(0, "/opt/trn_rl_repo")

import contextlib

import numpy as np

import concourse.bacc as bacc
import concourse.mybir as mybir
import concourse.tile as tile
from concourse.bass_utils import run_bass_kernel_spmd

B, Q, K, H, DV = 16, 256, 256, 256, 256
NCORES = 8
BPC = B // NCORES  # batches per core

F32 = mybir.dt.float32
F16 = mybir.dt.float16
Tanh = mybir.ActivationFunctionType.Tanh
Exp = mybir.ActivationFunctionType.Exp


def build_nc(nq=32, s_bufs=4, f_bufs=4, q_outer=True, ramp=(8, 8, 16), pool_every=0, big_from_b=None):
    nc = bacc.Bacc("TRN2", target_bir_lowering=False, debug=False, num_devices=1)

    # queries/keys/W are consumed in fp16: the projection matmul then
    # avoids the fp32 LOW/HIGH double-pass on the PE (2x fewer PE cycles)
    # and the input DMAs halve. PSUM accumulation stays fp32.
    qsT = nc.dram_tensor("qsT", [BPC, H, Q], F16, kind="ExternalInput").ap()
    ksT = nc.dram_tensor("ksT", [BPC, H, K], F16, kind="ExternalInput").ap()
    vals = nc.dram_tensor("vals", [BPC, K, DV], F32, kind="ExternalInput").ap()
    # W_q.T and W_k.T side by side -> one const DMA (the serialized SWDGE
    # const loads gate the first projection)
    Wcat = nc.dram_tensor("Wcat", [H, 2 * H], F16, kind="ExternalInput").ap()
    wvp = nc.dram_tensor("wvp", [128, 2 * 32 * 32], F16, kind="ExternalInput").ap()
    ident = nc.dram_tensor("ident", [128, 128], F32, kind="ExternalInput").ap()
    out = nc.dram_tensor("out", [BPC, Q, DV], F32, kind="ExternalOutput").ap()

    with tile.TileContext(nc) as tc, contextlib.ExitStack() as ctx:
        cpool = ctx.enter_context(tc.tile_pool(name="cpool", bufs=1))
        xin = ctx.enter_context(tc.tile_pool(name="xin", bufs=2))
        projsb = ctx.enter_context(tc.tile_pool(name="projsb", bufs=2))
        spool = ctx.enter_context(tc.tile_pool(name="spool", bufs=s_bufs))
        fpool = ctx.enter_context(tc.tile_pool(name="fpool", bufs=f_bufs))
        smpool = ctx.enter_context(tc.tile_pool(name="smpool", bufs=2))
        projps = ctx.enter_context(tc.tile_pool(name="projps", bufs=2, space="PSUM"))
        scoreps = ctx.enter_context(tc.tile_pool(name="scoreps", bufs=2, space="PSUM"))
        transps = ctx.enter_context(tc.tile_pool(name="transps", bufs=2, space="PSUM"))
        outps = ctx.enter_context(tc.tile_pool(name="outps", bufs=2, space="PSUM"))

        # constants — issued via GPSIMD SWDGE so the SP HWDGE issue slot
        # (~600ns per DMA of issuing-engine time) stays free for the batch
        # loads during the kernel head.
        W_sb = cpool.tile([128, 4 * H], F16, name="W_sb")

        def load_Wcat():
            # SP HWDGE, emitted between qin(b0) and kin(b0): the first
            # projection needs qin+Wcat, so this ordering lands the whole
            # chain ~3us earlier than the SWDGE path.
            nc.sync.dma_start(
                W_sb[:].rearrange("p (d c) -> p d c", c=2 * H),
                Wcat.rearrange("(d p) c -> p d c", p=128),
            )
        wvp_sb = cpool.tile([128, 2 * 32 * 32], F16, name="wvp_sb")
        nc.gpsimd.dma_start(wvp_sb[:], wvp[:])
        ident_sb = cpool.tile([128, 128], F32, name="ident_sb")
        nc.gpsimd.dma_start(ident_sb[:], ident[:])

        def load_one(xname, xap, b):
            t = xin.tile([128, 2 * 256], F16, name=f"{xname}in", tag=f"{xname}in")
            nc.sync.dma_start(
                t[:].rearrange("p (d f) -> p d f", f=256),
                xap[b].rearrange("(d p) f -> p d f", p=128),
            )
            return t

        def load_batch(b):
            xt = {"q": load_one("q", qsT, b)}
            if b == 0:
                load_Wcat()
            xt["k"] = load_one("k", ksT, b)
            return xt

        def load_values(b):
            t = xin.tile([128, 2 * DV], F32, name="vals", tag="vals")
            nc.sync.dma_start(
                t[:].rearrange("p (kh f) -> p kh f", f=DV),
                vals[b].rearrange("(kh p) f -> p kh f", p=128),
            )
            return t

        def projections(xt):
            """-> proj[xname, hh] = [h=128, seq=256] SBUF tiles.
            hh-major emission so the first tanh block unblocks early."""
            proj = {}
            for hh in range(2):
                for xname in ("q", "k"):
                    pp = projps.tile([128, 256], F32, name="pp", tag="pp")
                    xoff = 0 if xname == "q" else H
                    for d in range(2):
                        nc.tensor.matmul(
                            pp[:],
                            lhsT=W_sb[
                                :, d * 2 * H + xoff + hh * 128 : d * 2 * H + xoff + (hh + 1) * 128
                            ],
                            rhs=xt[xname][:, d * 256 : (d + 1) * 256],
                            start=(d == 0),
                            stop=(d == 1),
                        )
                    # q feeds the per-partition scalar operand (must be
                    # fp32); k is the streamed operand (fp16 for DVE 2x).
                    sb = projsb.tile(
                        [128, 256],
                        F32 if xname == "q" else F16,
                        name=f"{xname}T{hh}",
                        tag=f"{xname}T{hh}",
                    )
                    nc.vector.tensor_copy(sb[:], pp[:])
                    proj[xname, hh] = sb
            return proj

        def score_block(proj, ps_s, q0, sz, hh):
            """tanh + w_v reduction for q in [q0, q0+sz), one h half."""
            kT = proj["k", hh]
            qT = proj["q", hh]
            S = spool.tile([128, sz * K], F16, name="S", tag="S")
            for j in range(sz):
                q = q0 + j
                # spread a fraction of the broadcast-adds onto the otherwise
                # idle GPSIMD (Pool) engine so S production outruns ACT
                eng = nc.gpsimd if (pool_every and j % pool_every == pool_every - 1) else nc.vector
                eng.tensor_scalar_add(
                    S[:, j * K : (j + 1) * K], kT[:], qT[:, q : q + 1]
                )
            F = fpool.tile([128, sz * K], F16, name="F", tag="F")
            nc.scalar.activation(F[:], S[:], Tanh)
            for j in range(sz):
                q = q0 + j
                qh, cg, r = q // 128, (q % 128) // 32, q % 32
                nc.tensor.matmul(
                    ps_s[qh][cg * 32 : (cg + 1) * 32, :],
                    lhsT=wvp_sb[:, (hh * 32 + r) * 32 : (hh * 32 + r + 1) * 32],
                    rhs=F[:, j * K : (j + 1) * K],
                    start=(hh == 0 and r == 0),
                    stop=(hh == 1 and r == 31),
                    skip_group_check=True,
                    tile_position=(0, cg * 32),
                )

        def softmax_av(ps_s, vals_sb, b, qh, out_sb):
            exp_sb = smpool.tile([128, K], F32, name="exp_sb", tag="exp")
            den = smpool.tile([128, 1], F32, name="den", tag="den")
            nc.scalar.activation(exp_sb[:], ps_s[qh][:], Exp, accum_out=den[:])
            rec = smpool.tile([128, 1], F32, name="rec", tag="rec")
            nc.vector.reciprocal(rec[:], den[:])
            eTs = []
            for kh in range(2):
                pt = transps.tile([128, 128], F32, name="pt", tag="pt")
                nc.tensor.transpose(
                    pt[:], exp_sb[:, kh * 128 : (kh + 1) * 128], ident_sb[:]
                )
                eT = smpool.tile([128, 128], F32, name=f"eT{kh}", tag=f"eT{kh}")
                nc.vector.tensor_copy(eT[:], pt[:])
                eTs.append(eT)
            po = outps.tile([128, DV], F32, name="po", tag="po")
            for kh in range(2):
                nc.tensor.matmul(
                    po[:],
                    lhsT=eTs[kh][:],
                    rhs=vals_sb[:, kh * DV : (kh + 1) * DV],
                    start=(kh == 0),
                    stop=(kh == 1),
                    skip_group_check=True,
                )
            nc.vector.tensor_scalar_mul(
                out_sb[:, qh * DV : (qh + 1) * DV], po[:], rec[:, 0:1]
            )
            if qh == 1:
                # one DMA per batch for both q-halves (fewer SP issues/sems)
                nc.sync.dma_start(
                    out[b].rearrange("(qh p) f -> p qh f", p=128),
                    out_sb[:].rearrange("p (qh f) -> p qh f", f=DV),
                )

        def qh_blocks(b, qh):
            """Block sizes covering this q-half. The very first blocks of
            the kernel ramp up small so the first tanh issues ASAP."""
            head = list(ramp) if (b == 0 and qh == 0 and ramp) else []
            # mirror the ramp at the very end of the kernel: the final
            # block's PE matmul chain + exp is pure tail, so keep it short.
            tail = list(reversed(ramp)) if (b == BPC - 1 and qh == 1 and ramp) else []
            # later batches run double-size blocks: by then the DVE has
            # built enough S-production surplus to feed them, and each
            # halved ACTIVATE count saves ~280ns of ACT overhead.
            blk = 2 * nq if big_from_b is not None and b >= big_from_b else nq
            rest = 128 - sum(head) - sum(tail)
            assert rest >= 0
            sizes = head
            while rest:
                sz = min(blk, rest)
                sizes.append(sz)
                rest -= sz
            return sizes + tail

        for b in range(BPC):
            xt = load_batch(b)
            proj = projections(xt)
            vals_sb = load_values(b)
            ps_s = []
            for qh in range(2):
                ps_s.append(scoreps.tile([128, K], F32, name=f"ps_s{qh}", tag="sc"))
            out_sb = smpool.tile([128, 2 * DV], F32, name="out_sb", tag="osb")
            if q_outer:
                # q-block outer, h-half inner: column groups finalize as we
                # go; each q-half's softmax+AV is emitted as soon as its
                # blocks are done.
                # start/stop are additionally gated on r==0 / r==31 inside
                # score_block, so per-block flags only carry the h-half.
                for qh in range(2):
                    q0 = qh * 128
                    for sz in qh_blocks(b, qh):
                        for hh in range(2):
                            score_block(proj, ps_s, q0, sz, hh)
                        q0 += sz
                    softmax_av(ps_s, vals_sb, b, qh, out_sb)
            else:
                for hh in range(2):
                    for qb in range(Q // nq):
                        score_block(proj, ps_s, qb * nq, nq, hh)
                for qh in range(2):
                    softmax_av(ps_s, vals_sb, b, qh, out_sb)

    nc.compile()
    return nc


_nc_cache = None


def _get_nc():
    global _nc_cache
    if _nc_cache is None:
        _nc_cache = build_nc()
    return _nc_cache


def make_in_maps(queries, keys, values, W_q, W_k, w_v):
    qsT = np.ascontiguousarray(queries.transpose(0, 2, 1)).astype(np.float16)
    ksT = np.ascontiguousarray(keys.transpose(0, 2, 1)).astype(np.float16)
    values = np.ascontiguousarray(values).astype(np.float32)
    Wcat = np.ascontiguousarray(
        np.concatenate([np.asarray(W_q).T, np.asarray(W_k).T], axis=1)
    ).astype(np.float16)
    wvp = np.zeros((128, 2, 32, 32), np.float16)
    for hh in range(2):
        chunk = np.asarray(w_v[hh * 128 : (hh + 1) * 128]).astype(np.float16)
        for r in range(32):
            wvp[:, hh, r, r] = chunk
    wvp = np.ascontiguousarray(wvp.reshape(128, 2048))
    ident = np.eye(128, dtype=np.float32)
    maps = []
    for c in range(NCORES):
        sl = slice(c * BPC, (c + 1) * BPC)
        maps.append(
            dict(
                qsT=qsT[sl],
                ksT=ksT[sl],
                vals=values[sl],
                Wcat=Wcat,
                wvp=wvp,
                ident=ident,
            )
        )
    return maps


def kernel(queries, keys, values, W_q, W_k, w_v):
    nc = _get_nc()
    maps = make_in_maps(queries, keys, values, W_q, W_k, w_v)
    res = run_bass_kernel_spmd(nc, maps, core_ids=list(range(NCORES)))
    return np.concatenate(
        [res.results[c]["out"] for c in range(NCORES)], axis=0
    ).astype(np.float32)
